# revision 6
# baseline (speedup 1.0000x reference)
"""Distributed Trainium2 Bass kernel for a single attention head.

Reference computation (fp32 jax):
    q = queries @ Wq.T + bq        # [B,S,Df]
    k = keys    @ Wk.T + bk
    v = values  @ Wv.T + bv
    attn = softmax((q @ k.T) / sqrt(Df), axis=-1)
    out  = attn @ v                # [B,S,Df]

with B=4, S=4096, D_MODEL=1024, D_FEATURE=64.

Sharding: 8 cores = (batch b in 0..3) x (query-half h in 0..1).
Core c handles batch b=c//2, q rows [h*2048, (h+1)*2048). Each core gets
its q-half plus the FULL keys/values of its batch (no collectives), all
pre-transposed on the host to m-contraction-major layout and converted
to bf16 so matmuls run at full PE rate and DMA bytes are halved.

Kernel structure (per core):
  - inputs arrive i-block-major: [128, nblk * (8 m-chunks * 512 cols)]
    so each 512-column projection block is one contiguous 1MB DMA and
    projection of block i can start as soon as its DMA lands.
  - projections: psum[64, 512] accumulated over 8 m-chunks,
    lhsT = wT chunk [128, 64], rhs = xT chunk [128, 512]; DVE evicts
    psum -> bf16 SBUF with the per-feature bias added.
  - scores are computed TRANSPOSED, flash-style: ST[j, i]
    (lhsT = kT[64, jc*128:...], rhs = qT[64, i-chunk]) so softmax-exp
    input and the attn@v moving operand are both natural layout.
  - attention runs in TWO i-passes of 1024 q-rows each so that the
    scores psum can double-buffer (2x[128,1024] = 4 banks) next to the
    out.T accumulator ([65,1024] = 2 banks) within the 8 PSUM banks.
    Pass A is interleaved with the k/v projections at k-block
    granularity so the ACT engine starts exp-ing ~8us into the kernel.
  - exp is fused with the 1/8 score scale on ACT; output PT is bf16.
  - attn@v accumulates out.T[f, i] with lhsT = v_aug[j, 65] (v in
    natural [j, f] layout + ones column -> row 64 of out.T is the
    softmax denominator for free).
  - finals per pass: evict out.T, PE-transpose 128-row chunks,
    reciprocal of the denominator column, scale, DMA out fp32 rows.
"""

import numpy as np
import ml_dtypes

import concourse.bass as bass
import concourse.mybir as mybir
import concourse.tile as tile
from concourse import bacc
from concourse.bass_utils import run_bass_kernel_spmd
from concourse.masks import make_identity

B = 4
S = 4096
DM = 1024
DF = 64
NCORES = 8
SQ = S // 2          # local q rows per core
MC = DM // 128       # 8 contraction chunks
NI = 512             # moving-operand tile (one PSUM bank of fp32)
JC = S // 128        # 32 key chunks
NBQ = SQ // NI       # 4 q column blocks
NBK = S // NI        # 8 k/v column blocks
IP = SQ // 2         # 1024: i-rows per attention pass
BF16 = mybir.dt.bfloat16
F32 = mybir.dt.float32
NP_BF16 = ml_dtypes.bfloat16
EXP = mybir.ActivationFunctionType.Exp


def build_kernel(tc):
    nc = tc.nc
    xq = nc.dram_tensor("xq", [128, NBQ * MC * NI], BF16, kind="ExternalInput")
    xk = nc.dram_tensor("xk", [128, NBK * MC * NI], BF16, kind="ExternalInput")
    xv = nc.dram_tensor("xv", [128, NBK * MC * NI], BF16, kind="ExternalInput")
    wT = nc.dram_tensor("wT", [128, MC * 3 * DF], BF16, kind="ExternalInput")
    bias = nc.dram_tensor("bias", [DF, 3], F32, kind="ExternalInput")
    out = nc.dram_tensor("out", [SQ, DF], F32, kind="ExternalOutput")

    from contextlib import ExitStack

    with ExitStack() as ctx:
        const_pool = ctx.enter_context(tc.tile_pool(name="const", bufs=1))
        xin_pool = ctx.enter_context(tc.tile_pool(name="xin", bufs=16))
        act_pool = ctx.enter_context(tc.tile_pool(name="act", bufs=1))
        pt_pool = ctx.enter_context(tc.tile_pool(name="pt", bufs=4))
        outT_pool = ctx.enter_context(tc.tile_pool(name="outT", bufs=1))
        fin_pool = ctx.enter_context(tc.tile_pool(name="fin", bufs=2))
        # PSUM budget (8 banks): ppsum 2 (shared proj/vtrans/finals),
        # spsum 4, opsum 2.
        ppsum = ctx.enter_context(tc.tile_pool(name="ppsum", bufs=2, space="PSUM"))
        spsum = ctx.enter_context(tc.tile_pool(name="spsum", bufs=2, space="PSUM"))
        opsum = ctx.enter_context(tc.tile_pool(name="opsum", bufs=1, space="PSUM"))

        # ---- constants (DMA'd first) ----
        wT_sb = const_pool.tile([128, MC * 3 * DF], BF16, tag="wt")
        nc.sync.dma_start(wT_sb[:], wT[:])
        bias_sb = const_pool.tile([DF, 3], F32, tag="bias")
        nc.sync.dma_start(bias_sb[:], bias[:])
        # preload the ACT exp table while DMAs stream
        scratch = const_pool.tile([DF, 1], F32, tag="scratch")
        nc.scalar.activation(scratch[:], bias_sb[:, 0:1], EXP)
        ident = const_pool.tile([128, 128], BF16, tag="ident")
        make_identity(nc, ident[:])
        identf = const_pool.tile([128, 128], F32, tag="identf")
        make_identity(nc, identf[:])

        # ---- input DMAs, interleaved q first then k/v alternating ----
        def load_block(x_dram, i):
            t = xin_pool.tile([128, MC * NI], BF16, tag="xin")
            nc.sync.dma_start(t[:], x_dram[:, i * MC * NI:(i + 1) * MC * NI])
            return t

        q_tiles = [load_block(xq, i) for i in range(NBQ)]
        kv_tiles = {}
        for i in range(NBK):
            kv_tiles[("k", i)] = load_block(xk, i)
            kv_tiles[("v", i)] = load_block(xv, i)

        # ---- persistent activations ----
        qT_sb = act_pool.tile([DF, SQ], BF16, tag="qT")            # [64, 2048]
        kT_sb = act_pool.tile([DF, S], BF16, tag="kT")             # [64, 4096]
        vT_sb = act_pool.tile([DF, S], BF16, tag="vT")
        v_sb = act_pool.tile([128, JC * (DF + 1)], BF16, tag="v")  # [128, 32*65]
        nc.gpsimd.memset(v_sb[:], 1.0)  # col DF of every block stays 1.0

        def w_slice(mc_i, which):
            o = mc_i * 3 * DF + which * DF
            return wT_sb[:, o:o + DF]

        def project_block(x_tile, i, which, dest_sb, bias_col):
            """One 512-column projection block accumulated over 8 m-chunks."""
            ps = ppsum.tile([DF, NI], F32, tag="ps")
            for mc_i in range(MC):
                nc.tensor.matmul(
                    ps[:], w_slice(mc_i, which), x_tile[:, mc_i * NI:(mc_i + 1) * NI],
                    start=(mc_i == 0), stop=(mc_i == MC - 1),
                )
            nc.vector.tensor_scalar_add(
                dest_sb[:, i * NI:(i + 1) * NI], ps[:], bias_sb[:, bias_col:bias_col + 1])

        # ---- q projection up front ----
        for i in range(NBQ):
            project_block(q_tiles[i], i, 0, qT_sb, 0)

        def attn_chunk(jc, ipass, po):
            """Scores + exp + attn@v for one 128-row key chunk, one i-pass."""
            io = ipass * IP
            ss = spsum.tile([128, IP], F32, tag="ss")
            for ii in range(IP // NI):
                nc.tensor.matmul(
                    ss[:, ii * NI:(ii + 1) * NI],
                    kT_sb[:, jc * 128:(jc + 1) * 128],
                    qT_sb[:, io + ii * NI:io + (ii + 1) * NI],
                    start=True, stop=True,
                )
            pts = pt_pool.tile([128, IP], BF16, tag="pt")
            nc.scalar.activation(pts[:], ss[:], EXP, scale=0.125)
            for ii in range(IP // NI):
                nc.tensor.matmul(
                    po[:, ii * NI:(ii + 1) * NI],
                    v_sb[:, jc * (DF + 1):(jc + 1) * (DF + 1)],
                    pts[:, ii * NI:(ii + 1) * NI],
                    start=(jc == 0), stop=(jc == JC - 1),
                )

        def finals(ipass, po):
            """Evict out.T accumulator, transpose, normalize, store."""
            outT_sb = outT_pool.tile([DF + 1, IP], F32, tag="outT")
            nc.vector.tensor_copy(outT_sb[:], po[:])
            for c in range(IP // 128):
                pf = ppsum.tile([128, DF + 1], F32, tag="ps")
                nc.tensor.transpose(
                    pf[:], outT_sb[:, c * 128:(c + 1) * 128],
                    identf[0:DF + 1, 0:DF + 1])
                rcp = fin_pool.tile([128, 1], F32, tag="rcp")
                nc.vector.reciprocal(rcp[:], pf[:, DF:DF + 1])
                ob = fin_pool.tile([128, DF], F32, tag="ob")
                nc.vector.tensor_scalar_mul(ob[:], pf[:, 0:DF], rcp[:])
                r0 = ipass * IP + c * 128
                nc.sync.dma_start(out[r0:r0 + 128, :], ob[:])

        # ---- pass A: k/v projection interleaved with attention i<1024 ----
        poA = opsum.tile([DF + 1, IP], F32, tag="po")
        for kb in range(NBK):
            project_block(kv_tiles[("k", kb)], kb, 1, kT_sb, 1)
            project_block(kv_tiles[("v", kb)], kb, 2, vT_sb, 2)
            for jc in range(4 * kb, 4 * kb + 4):
                pv = ppsum.tile([128, DF], BF16, tag="ps")
                nc.tensor.transpose(
                    pv[:], vT_sb[:, jc * 128:(jc + 1) * 128], ident[0:DF, 0:DF])
                nc.vector.tensor_copy(
                    v_sb[:, jc * (DF + 1):jc * (DF + 1) + DF], pv[:])
            for jc in range(4 * kb, 4 * kb + 4):
                attn_chunk(jc, 0, poA)
        finals(0, poA)

        # ---- pass B: attention i in [1024, 2048) ----
        poB = opsum.tile([DF + 1, IP], F32, tag="po")
        for jc in range(JC):
            attn_chunk(jc, 1, poB)
        finals(1, poB)


_COMPILED = None


def get_compiled():
    global _COMPILED
    if _COMPILED is None:
        nc = bacc.Bacc("TRN2", target_bir_lowering=False, debug=False,
                       enable_asserts=False, num_devices=NCORES)
        with tile.TileContext(nc) as tc:
            build_kernel(tc)
        nc.compile()
        _COMPILED = nc
    return _COMPILED


def _to_block_major(xT):
    """[DM, s_len] -> [128, nblk*MC*NI]: 512-col blocks, m-chunk-major inside."""
    s_len = xT.shape[1]
    nblk = s_len // NI
    # (mc, p, blk, s) -> (p, blk, mc, s)
    return np.ascontiguousarray(
        xT.reshape(MC, 128, nblk, NI).transpose(1, 2, 0, 3).reshape(128, nblk * MC * NI))


def make_in_maps(queries, keys, values, Wq, bq, Wk, bk, Wv, bv):
    queries = np.asarray(queries, dtype=np.float32)
    keys = np.asarray(keys, dtype=np.float32)
    values = np.asarray(values, dtype=np.float32)
    wT_full = np.concatenate(
        [np.asarray(Wq).T, np.asarray(Wk).T, np.asarray(Wv).T], axis=1)  # [DM, 192]
    wT_host = np.ascontiguousarray(
        wT_full.reshape(MC, 128, 3 * DF).transpose(1, 0, 2).reshape(128, MC * 3 * DF)
    ).astype(NP_BF16)
    bias_host = np.stack(
        [np.asarray(bq), np.asarray(bk), np.asarray(bv)], axis=1
    ).astype(np.float32)

    in_maps = []
    for c in range(NCORES):
        b, h = c // 2, c % 2
        in_maps.append({
            "xq": _to_block_major(queries[b, h * SQ:(h + 1) * SQ, :].T).astype(NP_BF16),
            "xk": _to_block_major(keys[b].T).astype(NP_BF16),
            "xv": _to_block_major(values[b].T).astype(NP_BF16),
            "wT": wT_host, "bias": bias_host,
        })
    return in_maps


def assemble(results):
    out = np.zeros((B, S, DF), dtype=np.float32)
    for c in range(NCORES):
        b, h = c // 2, c % 2
        out[b, h * SQ:(h + 1) * SQ, :] = results[c]["out"]
    return out


def kernel(**inputs):
    nc = get_compiled()
    in_maps = make_in_maps(**inputs)
    res = run_bass_kernel_spmd(nc, in_maps, core_ids=list(range(NCORES)))
    return assemble(res.results)
